# revision 1
# baseline (speedup 1.0000x reference)
"""Trainium2 Bass kernel for the BalancedSpikingNetwork problem.

Strategy: model-parallel over neurons across 8 NeuronCores.
  - Each core owns 256 E-neurons + 64 I-neurons (padded to 384 = 3x128 rows).
  - Per step: 24 gathered spike chunks + 1 local input chunk accumulate into a
    PSUM tile [64, 320] = tau-scaled input currents for this core's neurons
    (batch-major). Weights are pre-scaled by tau on the host.
  - Recurrent matmuls run in float32r (single-pass fp32, exact for 0/1
    spikes); spikes cross cores as fp8 (0/1 is exact), upconverted to f32r
    in 4 ACT-engine chunks that pipeline with the matmul burst.
  - Spike exchange: AllToAll with an 8x-replicated input slab. On this
    runtime an 8-rank AllGather lowers to 3-stage RDH (~12.7us/call) while
    the same gather expressed as AllToAll runs single-phase Mesh
    (~7.3us/call); the replication costs one SBUF->DRAM DMA with a
    stride-0 free dim (the broadcast dim must sit inside the free dims,
    not outside the partition dim).
    (Direct SBUF->SBUF remote_dma P2P would be faster still, but its Q7
    ext-isa ucode library is absent from this runtime and crashes the
    device.)
  - LIF update with fused scalar_tensor_tensor ops on DVE; spike-rate sums
    accumulate in [batch, neuron] layout; final readout matmul on host.

The spike at step t depends only on state through t-1 (v_dec = v + dt*(i - v)
is computed before the step-t input current lands), so the exchange of z(t)
overlaps with the step-t matmuls.
"""

import os
import sys

for _p in ("/opt/trn_rl_repo", "/root/.axon_site/_ro/trn_rl_repo"):
    if _p not in sys.path:
        sys.path.append(_p)

import numpy as np
import ml_dtypes

import concourse.bass as bass
import concourse.mybir as mybir
import concourse.tile as tile
from concourse import bacc
from concourse.bass_utils import run_bass_kernel_spmd
from concourse.masks import make_identity

F32 = mybir.dt.float32
F32R = mybir.dt.float32r
BF16 = mybir.dt.bfloat16
FP8 = mybir.dt.float8e4
OP = mybir.AluOpType
ACT_COPY = mybir.ActivationFunctionType.Copy

EXC = os.environ.get("EXC", "fp8")       # fp8 | f32r  (exchange payload dtype)
EXC_DT = {"fp8": FP8, "f32r": F32R}[EXC]
FILL = int(os.environ.get("FILL", "0"))   # warmer matmuls per step
COMM = os.environ.get("COMM", "a2a")     # a2a | ag  (collective kind)

B, T_FULL, IN = 64, 512, 128
N_E, N_I = 2048, 512
NCORES = 8
E_LOC = N_E // NCORES          # 256
I_LOC = N_I // NCORES          # 64
NLOC = E_LOC + I_LOC           # 320 real outputs per core
PADLOC = 384                   # padded to 3 chunks of 128
NCHUNK = PADLOC // 128         # 3 chunks per source core
GCHUNK = NCORES * NCHUNK       # 24 gathered spike chunks
KSRC = GCHUNK * 128            # 3072 gathered contraction rows
SLOT = NCHUNK * B              # 192 staging columns per core

TAU_E = 1.0 / 20.0
TAU_I = 1.0 / 10.0
SYN_DEC = 1.0 - 1.0 / 5.0      # 0.8


def build_kernel(T: int):
    nc = bacc.Bacc(
        "TRN2", target_bir_lowering=False, debug=False, num_devices=NCORES
    )

    W_in = nc.dram_tensor("W", [KSRC, NLOC], F32R, kind="ExternalInput")
    WIN_in = nc.dram_tensor("WIN", [IN, NLOC], F32, kind="ExternalInput")
    XT_in = nc.dram_tensor("XT", [T, IN, B], F32, kind="ExternalInput")
    RATES_out = nc.dram_tensor("RATES", [B, NLOC], F32, kind="ExternalOutput")

    rg = [list(range(NCORES))]

    with tile.TileContext(nc) as tc:
        with (
            tc.tile_pool(name="persist", bufs=1) as pp,
            tc.tile_pool(name="step", bufs=2) as sp,
            tc.tile_pool(name="psum", bufs=2, space="PSUM") as psp,
            tc.tile_pool(name="tpsum", bufs=1, space="PSUM") as tpp,
            tc.tile_pool(name="dram", bufs=2, space="DRAM") as dp,
        ):
            # --- persistent tiles ---
            w_sb = pp.tile([128, GCHUNK * NLOC], F32R)            # recurrent wts
            win_sb = pp.tile([128, NLOC], F32)                    # input weights
            v_sb = pp.tile([B, NLOC], F32)                        # membrane
            u_sb = pp.tile([B, NLOC], F32)                        # tau*syn current
            zt_sb = pp.tile([128, SLOT], EXC_DT)                  # spikes [n, b]
            rates_sb = pp.tile([B, NLOC], F32)                    # counts [b, n]
            ident = pp.tile([B, B], F32)

            for k in range(GCHUNK):
                nc.sync.dma_start(
                    out=w_sb[:, k * NLOC : (k + 1) * NLOC],
                    in_=W_in[k * 128 : (k + 1) * 128, :],
                )
            nc.sync.dma_start(out=win_sb, in_=WIN_in[:])
            make_identity(nc, ident)
            nc.vector.memset(v_sb, 0.0)
            nc.vector.memset(u_sb, 0.0)
            nc.vector.memset(rates_sb, 0.0)

            ag_prev = None   # gathered spikes of step t-1
            psum_prev = None  # currents computed at step t-1
            s_t_prev = None  # upconverted spikes of step t-1 (warmer input)

            for t in range(T):
                # ---- v_dec(t) = a*v(t-1) + 0.8*u(t-2) + psum(t-1).
                # t1 = a*v + 0.8*u uses only older state, so it overlaps the
                # previous burst; psum lands via ONE tensor_tensor add. ----
                u08 = sp.tile([B, NLOC], F32, tag="U8")
                nc.vector.tensor_scalar(
                    out=u08, in0=u_sb, scalar1=SYN_DEC, scalar2=None, op0=OP.mult
                )
                t1 = sp.tile([B, NLOC], F32, tag="T1")
                nc.vector.scalar_tensor_tensor(
                    out=t1[:, :E_LOC], in0=v_sb[:, :E_LOC], scalar=1.0 - TAU_E,
                    in1=u08[:, :E_LOC], op0=OP.mult, op1=OP.add,
                )
                nc.vector.scalar_tensor_tensor(
                    out=t1[:, E_LOC:], in0=v_sb[:, E_LOC:], scalar=1.0 - TAU_I,
                    in1=u08[:, E_LOC:], op0=OP.mult, op1=OP.add,
                )
                v_dec = sp.tile([B, NLOC], F32, tag="VD")
                if psum_prev is None:
                    nc.vector.tensor_copy(v_dec, t1)
                else:
                    nc.vector.tensor_tensor(
                        out=v_dec, in0=t1, in1=psum_prev, op=OP.add
                    )

                # ---- spikes in [n, b] layout: transpose + threshold ----
                for j in range(NCHUNK):
                    w = 128 if j < 2 else I_LOC
                    tp = tpp.tile([128, B], F32, tag=f"TP{j}")
                    nc.tensor.transpose(
                        tp[:w, :], v_dec[:, j * 128 : j * 128 + w], ident
                    )
                    # full 128 rows: pad rows get 0/1 garbage that multiplies
                    # zero weight columns (is_gt never yields NaN)
                    nc.vector.tensor_scalar(
                        out=zt_sb[:, j * B : (j + 1) * B], in0=tp[:, :],
                        scalar1=1.0, scalar2=None, op0=OP.is_gt,
                    )

                # ---- exchange spikes (overlaps the matmul burst below) ----
                if 1 <= t <= T - 3:
                    ag_out = dp.tile(
                        [NCORES * 128, SLOT], EXC_DT, tag="AGO",
                        addr_space="Local" if COMM == "a2a" else "Shared",
                    )
                    if COMM == "a2a":
                        # AllToAll with 8x-replicated input == AllGather,
                        # but single-phase (~5us) instead of 3-stage RDH
                        # (~12.7us) on this runtime.
                        a2a_in = dp.tile([NCORES * 128, SLOT], EXC_DT,
                                         tag="AGI")
                        # one-hop 8x replication: stride-0 FREE dim on
                        # the SBUF source (partition dim stays first)
                        nc.sync.dma_start(
                            out=a2a_in[:].rearrange("(d p) c -> p d c",
                                                    p=128),
                            in_=zt_sb[:].unsqueeze(1).broadcast_to(
                                [128, NCORES, SLOT]),
                        )
                        nc.gpsimd.collective_compute(
                            "AllToAll",
                            OP.bypass,
                            replica_groups=rg,
                            ins=[a2a_in[:]],
                            outs=[ag_out[:]],
                        )
                    else:
                        ag_in = dp.tile([128, SLOT], EXC_DT, tag="AGI")
                        nc.sync.dma_start(out=ag_in[:, : 2 * B],
                                          in_=zt_sb[:, : 2 * B])
                        nc.sync.dma_start(out=ag_in[:, 2 * B :],
                                          in_=zt_sb[:, 2 * B :])
                        nc.gpsimd.collective_compute(
                            "AllGather",
                            OP.bypass,
                            replica_groups=rg,
                            ins=[ag_in[:]],
                            outs=[ag_out[:]],
                        )
                    new_ag = ag_out
                else:
                    new_ag = None if t == 0 else ag_prev

                # ---- u(t-1) = 0.8*u(t-2) + psum(t-1), off the chain ----
                if psum_prev is not None:
                    nc.vector.tensor_tensor(
                        out=u_sb, in0=u08, in1=psum_prev, op=OP.add
                    )

                # ---- input currents for step t (consumed at t+1) ----
                if t < T - 1:
                    sx_t = sp.tile([128, B], F32, tag="SX")
                    nc.sync.dma_start(out=sx_t, in_=XT_in[t])
                    # PE warmers: cheap f32r matmuls with no exchange
                    # dependency keep the PE p-state clock up during the
                    # collective wait
                    for f in range(FILL if s_t_prev is not None else 0):
                        jp = psp.tile([B, NLOC], F32, tag="JP", bufs=1)
                        nc.tensor.matmul(
                            jp, s_t_prev[:, (f % GCHUNK) * B : (f % GCHUNK + 1) * B],
                            w_sb[:, :NLOC], start=True, stop=True)
                    psum = psp.tile([B, NLOC], F32, tag="PS")
                    nc.tensor.matmul(
                        psum, sx_t, win_sb, start=True, stop=(ag_prev is None)
                    )
                    if ag_prev is not None:
                        s_raw = sp.tile([128, GCHUNK * B], EXC_DT, tag="SR")
                        H = NCORES // 2
                        nc.sync.dma_start(
                            out=s_raw[:, : H * SLOT].rearrange(
                                "p (d c) -> p d c", d=H),
                            in_=ag_prev[: H * 128].rearrange(
                                "(d p) c -> p d c", p=128),
                        )
                        nc.scalar.dma_start(
                            out=s_raw[:, H * SLOT :].rearrange(
                                "p (d c) -> p d c", d=H),
                            in_=ag_prev[H * 128 :].rearrange(
                                "(d p) c -> p d c", p=128),
                        )
                        if EXC == "fp8":
                            s_t = sp.tile([128, GCHUNK * B], F32R, tag="S")
                            q = GCHUNK * B // 4
                            for i in range(4):
                                nc.scalar.activation(
                                    out=s_t[:, i * q : (i + 1) * q],
                                    in_=s_raw[:, i * q : (i + 1) * q],
                                    func=ACT_COPY,
                                )
                        else:
                            s_t = s_raw
                        s_t_prev = s_t
                        for k in range(GCHUNK):
                            nc.tensor.matmul(
                                psum,
                                s_t[:, k * B : (k + 1) * B],
                                w_sb[:, k * NLOC : (k + 1) * NLOC],
                                start=False,
                                stop=(k == GCHUNK - 1),
                            )
                else:
                    psum = None
                ag_prev = new_ag

                # ---- rates accumulation in [b, n] layout ----
                zbn = sp.tile([B, NLOC], F32, tag="ZB")
                nc.vector.tensor_scalar(
                    out=zbn, in0=v_dec, scalar1=1.0, scalar2=None, op0=OP.is_gt
                )
                nc.gpsimd.tensor_tensor(
                    out=rates_sb, in0=rates_sb, in1=zbn, op=OP.add
                )

                # ---- v(t) = (v_dec <= 1) * v_dec ----
                nc.vector.scalar_tensor_tensor(
                    out=v_sb, in0=v_dec, scalar=1.0, in1=v_dec,
                    op0=OP.is_le, op1=OP.mult,
                )
                psum_prev = psum

            nc.sync.dma_start(out=RATES_out[:], in_=rates_sb[:])

    nc.compile()
    return nc


def _prep_inputs(x, W_ee, W_ie, W_ei, W_ii, W_e_in, W_i_in):
    """Host-side: combined per-core weight matrices (tau-pre-scaled) +
    transposed input."""
    Wee = np.maximum(W_ee, 0).astype(np.float32)
    Wie = np.maximum(W_ie, 0).astype(np.float32)
    Wei = np.maximum(W_ei, 0).astype(np.float32)
    Wii = np.maximum(W_ii, 0).astype(np.float32)

    Ws, Wins = [], []
    for c in range(NCORES):
        Ec = slice(c * E_LOC, (c + 1) * E_LOC)
        Ic = slice(c * I_LOC, (c + 1) * I_LOC)
        Wc = np.zeros((KSRC, NLOC), np.float32)
        for d in range(NCORES):
            base = d * PADLOC
            Epre = slice(d * E_LOC, (d + 1) * E_LOC)
            Ipre = slice(d * I_LOC, (d + 1) * I_LOC)
            Wc[base : base + E_LOC, :E_LOC] = Wee[Ec, Epre].T
            Wc[base : base + E_LOC, E_LOC:] = Wie[Ic, Epre].T
            Wc[base + E_LOC : base + NLOC, :E_LOC] = -Wei[Ec, Ipre].T
            Wc[base + E_LOC : base + NLOC, E_LOC:] = -Wii[Ic, Ipre].T
        Wc[:, :E_LOC] *= TAU_E
        Wc[:, E_LOC:] *= TAU_I
        Ws.append(Wc)

        Wi = np.empty((IN, NLOC), np.float32)
        Wi[:, :E_LOC] = W_e_in[Ec].T * TAU_E
        Wi[:, E_LOC:] = W_i_in[Ic].T * TAU_I
        Wins.append(Wi)

    xT = np.ascontiguousarray(
        np.asarray(x, np.float32).transpose(1, 2, 0)
    )  # [T, IN, B]
    return Ws, Wins, xT


_CACHE = {}


def _get_kernel(T):
    if T not in _CACHE:
        _CACHE[T] = build_kernel(T)
    return _CACHE[T]


def run_spikes(x, W_ee, W_ie, W_ei, W_ii, W_e_in, W_i_in, T=None, trace=False):
    """Run the device portion; returns spike-count sums [B, N_E] and results."""
    T = x.shape[1] if T is None else T
    Ws, Wins, xT = _prep_inputs(x, W_ee, W_ie, W_ei, W_ii, W_e_in, W_i_in)
    xT = xT[:T]
    nc = _get_kernel(T)
    in_maps = [{"W": Ws[c], "WIN": Wins[c], "XT": xT} for c in range(NCORES)]
    res = run_bass_kernel_spmd(
        nc, in_maps, core_ids=list(range(NCORES)), trace=trace
    )
    R = np.stack([res.results[c]["RATES"] for c in range(NCORES)])  # [c, b, 320]
    counts = (
        R[:, :, :E_LOC].transpose(1, 0, 2).reshape(B, N_E)
    )  # [b, c*256 + n]
    return counts, res


def kernel(x, W_ee, W_ie, W_ei, W_ii, W_e_in, W_i_in, readout_w, readout_b):
    counts, _ = run_spikes(x, W_ee, W_ie, W_ei, W_ii, W_e_in, W_i_in)
    rates = counts / np.float32(x.shape[1])
    y = rates.astype(np.float32) @ np.asarray(readout_w, np.float32).T
    return (y + np.asarray(readout_b, np.float32)).astype(np.float32)

